# revision 15
# baseline (speedup 1.0000x reference)
"""Trainium2 Bass kernel for nn_Minerva_with_encoding (retrieval_knn).

Math (reference):
    pos_ids = argmin_j |R - enc_ids[j]|        [M]
    R_enc   = pos_encoding[pos_ids]            [M, 4]
    Xw = X @ Wx_w.T + Wx_b                     [N, 768]
    Dw = D @ Wd_w.T + Wd_b                     [M, 768]
    a  = Xw @ Dw.T                             [N, M]
    a  = sign(a) * |a|^2  ( = a * |a| )
    echo = a @ R_enc                           [N, 4]
    out  = echo @ We_w.T + We_b                [N, 1]

Strategy:
  * Host folds the two projections into one:  with A = Wx_w.T, B = Wd_w.T,
        a = X @ C @ D.T + p[n] + q[m] + c0
    where C = A @ B.T = Wx_w.T @ Wd_w   [768, 768]   (host, fp64)
          p = X @ (A @ Wd_b)  [N],  q = D @ (B @ Wx_b)  [M],  c0 = Wx_b.Wd_b.
    Raw D.T then streams straight into the score matmul — no on-device
    projection of D at all.
  * Host folds We into the encoding gather: v = R_enc @ We_w.T  [M, 1], so
    out = act(a) @ v + We_b.  argmin/gather (1M flops) runs on host.
  * Because D needs no projection, the optimal sharding is pure-N: each of
    the 8 cores takes a 512-query slab and the FULL exemplar set.  The
    per-core G projection (X-slab @ C) is 1/8 of the total G work — zero
    replicated compute.  Host output is a plain concat (+We_b).
  * Per core, transposed [feature-on-partitions] layout:
      GT [768, 512]  = C-tiles.T @ XT-tiles                (PE)
      aT tiles [128m, 512n] = DT-slices.T @ GT             (PE, PSUM fp32)
      s = a + q[m] + p[n]   (one DVE scalar_tensor_tensor pass)
      act = s * |s|         (ACT abs, DVE mult)
      partial[1, 512] += v_m.T @ act                       (PE reduction)
    D.T (12 MB in bf16) streams through SBUF in [128, 6, 512] blocks — ONE
    DMA per block, and the host packs the stream chunk-major so each block
    is a single contiguous 0.75 MB DRAM span (vs 768 scattered 1 KB runs;
    ~5 us faster per exec with 8 cores sharing HBM banks).
  * Per-core operands are packed into TWO DRAM inputs: a merged f32r block
    ([768, MW], column-packed xt|cm|qvp|tailr) plus the bf16 D.T stream, so
    each dispatch passes 3 buffer handles instead of 7 — lower per-exec
    marshaling through the axon relay.  enable_partition_id=False drops
    another handle (no collectives are used; cores differ only by input).
  * Score matmuls run bf16 x bf16 -> fp32 PSUM: G is cast to bf16 during
    the phase-A PSUM->SBUF copy and D.T is stored bf16 in DRAM, halving
    the dominant HBM stream (24 -> 12 MB/core; 8 cores stream concurrently
    against shared HBM) at the same PE rate (1 cycle/row for bf16 and f32r
    alike at 512-wide moving operand).  End-to-end max rel err ~2.5e-3 vs
    the 2e-2 gate.  The phase-A projection stays f32r.
  * Per-core PE work is 512*8192*768 (scores) + 512*768*768 (proj) MACs
    = 215k cycles ~ 90 us at 2.4 GHz — the compute roofline for this
    sharding; the ~13 MB/core HBM stream hides under it.  Measured
    steady-state: ~100-110 us per execution.

  _build_nc(reps=K) unrolls the whole body K times inside one NEFF; the
  K>1 builds exist only for benchmarking (test.py measures true on-device
  time as the slope d(wall)/d(reps), which cancels the multi-ms axon
  dispatch overhead).  kernel() always uses reps=1.
"""

import numpy as np
import ml_dtypes

import concourse.bacc as bacc
import concourse.mybir as mybir
import concourse.tile as tile
from concourse.bass_utils import run_bass_kernel_spmd

F32 = mybir.dt.float32
F32R = mybir.dt.float32r
BF16 = mybir.dt.bfloat16

N_CORES = 8
N_Q = 4096  # query rows
N_D = 8192  # exemplar rows
D_IN = 768  # input features
REP = 768  # projection features

N_SL = N_Q // N_CORES  # 512-query slab per core
M_SL = N_D  # full exemplar set per core

DT_TILES = D_IN // 128  # 6
RT_TILES = REP // 128  # 6 (output dim of C)
NCH = 512  # moving-chunk size (= N_SL)
M_TILES = M_SL // 128  # 64
MC_TOTAL = M_SL // NCH  # 16 D.T m-chunks to stream
WARMUP_MM = 4  # throwaway matmuls to warm the PE during the DMA fill

# merged-input column layout: [768, MW] f32r
QVP_W = 2 * M_TILES + N_SL + 1  # 641
TAIL_W = 2 * 128 + NCH + 128  # 896
XT_OFF = 0
CM_OFF = XT_OFF + N_SL  # 512
QVP_OFF = CM_OFF + REP  # 1280  (rows 0:128)
TAIL_OFF = QVP_OFF + QVP_W  # 1921 (row 0 only)
MW = TAIL_OFF + TAIL_W  # 2817

_CACHED = {}


def _build_nc(reps=1):
    nc = bacc.Bacc(
        "TRN2",
        target_bir_lowering=False,
        debug=False,
        num_devices=N_CORES,
        enable_partition_id=False,
    )
    mg = nc.declare_dram_parameter("mg", [D_IN, MW], F32R, isOutput=False)
    mgd = nc.declare_dram_parameter(
        "mgd", [MC_TOTAL * 128, DT_TILES, NCH], BF16, isOutput=False
    )
    partial = nc.declare_dram_parameter("partial", [1, N_SL], F32, isOutput=True)

    with tile.TileContext(nc) as tc:
        with (
            tc.tile_pool(name="cp", bufs=2) as cp,
            tc.tile_pool(name="gp", bufs=1) as gp,
            tc.tile_pool(name="srcp", bufs=2) as srcp,
            tc.tile_pool(name="dtp", bufs=6) as dtp,
            tc.tile_pool(name="smallp", bufs=1) as smallp,
            tc.tile_pool(name="actp", bufs=2) as actp,
            tc.tile_pool(name="pp", bufs=7, space="PSUM") as pp,
            tc.tile_pool(name="redp", bufs=1, space="PSUM") as redp,
        ):
            # PE warm-up: throwaway matmuls on scratch SBUF run inside the
            # initial DMA fill and lift the PE out of its cold P-state.
            warm_sb = smallp.tile([128, NCH], F32, tag="warm")
            nc.vector.memset(warm_sb, 0.0)
            warm_ps = pp.tile([128, NCH], F32, tag="big", name="warm_ps")
            for _ in range(WARMUP_MM):
                nc.tensor.matmul(
                    warm_ps, warm_sb[:, 0:128], warm_sb, start=True, stop=True
                )

            # 3D [partition, d-tile, col] views of the [768, *] column blocks:
            # one DMA moves a whole multi-tile block.
            xt3 = mg[:, XT_OFF : XT_OFF + N_SL].rearrange(
                "(t p) m -> p t m", p=128
            )
            cm3 = mg[:, CM_OFF : CM_OFF + REP].rearrange("(t p) m -> p t m", p=128)

            for _rep in range(reps):
                src_all = srcp.tile([128, DT_TILES, NCH], F32R, tag="src")
                nc.sync.dma_start(out=src_all, in_=xt3)
                c_all = cp.tile([128, DT_TILES, REP], F32R, tag="c")
                nc.sync.dma_start(out=c_all[:, :, 0:128], in_=cm3[:, :, 0:128])
                qvp_sb = smallp.tile([128, QVP_W], F32R, tag="qvp", bufs=2)
                for r in range(1, RT_TILES):
                    nc.sync.dma_start(
                        out=c_all[:, :, r * 128 : (r + 1) * 128],
                        in_=cm3[:, :, r * 128 : (r + 1) * 128],
                    )

                qb_sb = qvp_sb[:, 0:M_TILES]
                v_sb = qvp_sb[:, M_TILES : 2 * M_TILES]
                p_sb = qvp_sb[:, 2 * M_TILES : 2 * M_TILES + N_SL]
                tail_sb = smallp.tile([1, TAIL_W], F32R, tag="tailr", bufs=2)
                nc.sync.dma_start(
                    out=tail_sb, in_=mg[0:1, TAIL_OFF : TAIL_OFF + TAIL_W]
                )
                ones_sb = smallp.tile([128, 1], F32, tag="ones", bufs=2)
                nc.vector.memset(ones_sb, 1.0)

                # D.T streaming chunks, one DMA per [128, 6, 512] block
                dt_tiles = {}

                dt0_t = dtp.tile([128, DT_TILES, NCH], BF16, tag="dt", name="dt")

                def chunk_view(mc):
                    return mgd[mc * 128 : (mc + 1) * 128, :, :]

                def load_dt_mchunk(mc, halves=False):
                    t = dtp.tile(
                        [128, DT_TILES, NCH], BF16, tag="dt", name="dt"
                    )
                    cv = chunk_view(mc)
                    if halves:
                        nc.sync.dma_start(
                            out=t[:, :, 0 : NCH // 2],
                            in_=cv[:, :, 0 : NCH // 2],
                        )
                        nc.sync.dma_start(
                            out=t[:, :, NCH // 2 : NCH],
                            in_=cv[:, :, NCH // 2 : NCH],
                        )
                    else:
                        nc.sync.dma_start(out=t, in_=cv)
                    dt_tiles[mc] = t

                nc.sync.dma_start(
                    out=dt0_t[:, :, 0 : NCH // 2],
                    in_=mgd[0:128, :, 0 : NCH // 2],
                )
                nc.sync.dma_start(
                    out=qvp_sb, in_=mg[0:128, QVP_OFF : QVP_OFF + QVP_W]
                )
                nc.sync.dma_start(
                    out=dt0_t[:, :, NCH // 2 : NCH],
                    in_=mgd[0:128, :, NCH // 2 : NCH],
                )
                dt_tiles[0] = dt0_t

                g_sb = [
                    gp.tile([128, N_SL], BF16, tag=f"g{r}", name=f"g{r}")
                    for r in range(RT_TILES)
                ]

                # --- phase A: GT = C.T-tiles @ XT-tiles, r-pairs interleaved
                # (12 back-to-back matmuls per pair, 3 restarts instead of
                # 6); each pair's two G copies then run on DVE and ACT in
                # parallel while the next pair's matmuls issue.
                for rp in range(0, RT_TILES, 2):
                    ps_pair = [
                        pp.tile([128, NCH], F32, tag="big", name="proj_ps"),
                        pp.tile([128, NCH], F32, tag="big", name="proj_ps"),
                    ]
                    for d in range(DT_TILES):
                        for j in range(2):
                            nc.tensor.matmul(
                                ps_pair[j],
                                c_all[
                                    :, d, (rp + j) * 128 : (rp + j + 1) * 128
                                ],
                                src_all[:, d, :],
                                start=(d == 0),
                                stop=(d == DT_TILES - 1),
                                skip_group_check=True,
                            )
                    nc.vector.tensor_copy(g_sb[rp], ps_pair[0])
                    nc.scalar.copy(g_sb[rp + 1], ps_pair[1])

                # --- phase B: scores + corrections + power-sign + reduction --
                # v is folded into the activation (act' = (s*v[m]) * |s|), so
                # the m-reduction becomes an elementwise accumulation of act'
                # tiles.  That chain runs on the otherwise-idle GPSIMD engine,
                # hidden under the PE score matmuls; one final ones-stationary
                # matmul does the 128-partition reduction.
                mc_loaded = 1
                acc_t = None
                quad_ps = [None, None, None, None]
                for m in range(M_TILES):
                    if m % 4 == 0:
                        mc = m // 4
                        want = min(MC_TOTAL, mc + 5)
                        while mc_loaded < want:
                            load_dt_mchunk(mc_loaded, halves=(mc_loaded == 1))
                            mc_loaded += 1
                        # four m-tiles' accumulation groups interleave: all
                        # PSUM-recycle waits (and the dt chunk-ready wait —
                        # a quad consumes exactly one chunk) cluster at the
                        # group start, and the 24 matmuls issue back-to-back,
                        # quartering PE pipeline restarts vs one-per-tile.
                        # PSUM accumulation is per-bank, so the interleaved
                        # groups don't interact; the 7-deep "big" rotation
                        # keeps the previous quad draining while this one
                        # fills.
                        quad_ps = [
                            pp.tile([128, NCH], F32, tag="big", name="a_ps")
                            for _ in range(4)
                        ]
                        for r in range(RT_TILES):
                            for j in range(4):
                                nc.tensor.matmul(
                                    quad_ps[j],
                                    dt_tiles[mc][
                                        :, r, j * 128 : (j + 1) * 128
                                    ],
                                    g_sb[r],
                                    start=(r == 0),
                                    stop=(r == RT_TILES - 1),
                                    skip_group_check=True,
                                )
                    a_ps = quad_ps[m % 4]
                    # s = a + q[m] + p[n]  (single DVE pass, psum -> sbuf)
                    # act' = (s * v[m]) * |s|; the last m-tile runs half-width
                    # so the tail drains in ~half the latency
                    s_t = actp.tile([128, NCH], F32, tag="s", bufs=3, name="s_t")
                    abs_t = actp.tile(
                        [128, NCH], F32, tag="abs", bufs=3, name="abs_t"
                    )
                    act_t = actp.tile(
                        [128, NCH], F32, tag="act", bufs=3, name="act_t"
                    )
                    new_acc = actp.tile(
                        [128, NCH], F32, tag="acc", bufs=2, name="acc_t"
                    )
                    halves = 2 if m == M_TILES - 1 else 1
                    w = NCH // halves
                    for h in range(halves):
                        sl = slice(h * w, (h + 1) * w)
                        nc.vector.scalar_tensor_tensor(
                            s_t[:, sl],
                            in0=a_ps[:, sl],
                            scalar=qb_sb[:, m : m + 1],
                            in1=p_sb[:, sl],
                            op0=mybir.AluOpType.add,
                            op1=mybir.AluOpType.add,
                        )
                        nc.scalar.activation(
                            abs_t[:, sl],
                            s_t[:, sl],
                            mybir.ActivationFunctionType.Abs,
                        )
                        nc.vector.scalar_tensor_tensor(
                            act_t[:, sl],
                            in0=s_t[:, sl],
                            scalar=v_sb[:, m : m + 1],
                            in1=abs_t[:, sl],
                            op0=mybir.AluOpType.mult,
                            op1=mybir.AluOpType.mult,
                        )
                        # accumulate on GPSIMD (serial chain, ping-pong
                        # buffers); the last m-tile bypasses the chain and is
                        # reduced directly by its own final matmul
                        if m < M_TILES - 1:
                            if acc_t is None:
                                nc.gpsimd.tensor_copy(
                                    new_acc[:, sl], act_t[:, sl]
                                )
                            else:
                                nc.gpsimd.tensor_tensor(
                                    new_acc[:, sl],
                                    in0=acc_t[:, sl],
                                    in1=act_t[:, sl],
                                    op=mybir.AluOpType.add,
                                )
                    if m < M_TILES - 1:
                        acc_t = new_acc
                    else:
                        last_act = act_t

                # final 128-partition reduction: ones.T @ acc(m<=62) runs
                # while DVE is still producing act'(63); act'(63) halves
                # reduce last
                red_ps = redp.tile([1, NCH], F32, tag="red", name="red_ps")
                nc.tensor.matmul(red_ps, ones_sb, acc_t, start=True, stop=False)
                for h in range(2):
                    sl = slice(h * (NCH // 2), (h + 1) * (NCH // 2))
                    nc.tensor.matmul(
                        red_ps[:, sl],
                        ones_sb,
                        last_act[:, sl],
                        start=False,
                        stop=True,
                    )
                out_sb = actp.tile([1, NCH], F32, tag="out", bufs=1, name="out_sb")
                nc.scalar.copy(out_sb, red_ps)
                nc.sync.dma_start(out=partial[0:1, :], in_=out_sb)

    nc.compile()
    return nc


def _get_nc(reps=1):
    if ("nc", reps) not in _CACHED:
        _CACHED[("nc", reps)] = _build_nc(reps)
    return _CACHED[("nc", reps)]


def make_in_maps(inputs):
    X = np.asarray(inputs["X"], dtype=np.float32)
    D = np.asarray(inputs["D"], dtype=np.float32)
    R = np.asarray(inputs["R"], dtype=np.float32)
    Wx_w = np.asarray(inputs["Wx_w"], np.float32)
    Wd_w = np.asarray(inputs["Wd_w"], np.float32)
    Wx_b = np.asarray(inputs["Wx_b"], np.float32)
    Wd_b = np.asarray(inputs["Wd_b"], np.float32)

    # --- host: nearest-encoding lookup, fold We into v ----------------------
    pos_ids = np.argmin(
        np.abs(R - np.asarray(inputs["encoding_ids"], np.float32)[None, :]),
        axis=1,
    )
    R_enc = np.asarray(inputs["pos_encoding"], np.float32)[pos_ids]  # [M, R_DIM]
    v = (
        R_enc.astype(np.float64) @ np.asarray(inputs["We_w"], np.float64).T
    ).astype(np.float32)  # [M, 1]

    # --- host: fold the two projections (fp64) ------------------------------
    A64 = Wx_w.T.astype(np.float64)  # [d, r]
    B64 = Wd_w.T.astype(np.float64)  # [d', r]
    C = np.ascontiguousarray((A64 @ B64.T).astype(np.float32))  # [d, d']
    p = (X.astype(np.float64) @ (A64 @ Wd_b.astype(np.float64))).astype(
        np.float32
    )  # [N]
    q = D.astype(np.float64) @ (B64 @ Wx_b.astype(np.float64))  # [M] f64
    c0 = float(Wx_b.astype(np.float64) @ Wd_b.astype(np.float64))
    qc = (q + c0).astype(np.float32)  # [M]

    XT = np.ascontiguousarray(X.T)  # [768, 4096]
    DTm = np.ascontiguousarray(D.T)  # [768, 8192]
    # chunk-major repack: mgd[c*128+p, t, m] = D.T[t*128+p, c*512+m], so each
    # [128, 6, 512] streaming chunk is one contiguous 0.75 MB DRAM span
    DTm16 = np.ascontiguousarray(
        DTm.astype(ml_dtypes.bfloat16)
        .reshape(DT_TILES, 128, MC_TOTAL, NCH)
        .transpose(2, 1, 0, 3)
        .reshape(MC_TOTAL * 128, DT_TILES, NCH)
    )
    qbm = np.ascontiguousarray(qc.reshape(M_TILES, 128).T)  # [128, 64]
    vb = np.ascontiguousarray(v[:, 0].reshape(M_TILES, 128).T)  # [128, 64]

    in_maps = []
    for c in range(N_CORES):
        nsl = slice(c * N_SL, (c + 1) * N_SL)
        mg = np.zeros((D_IN, MW), np.float32)
        mg[:, XT_OFF : XT_OFF + N_SL] = XT[:, nsl]
        mg[:, CM_OFF : CM_OFF + REP] = C
        mg[0:128, QVP_OFF : QVP_OFF + M_TILES] = qbm
        mg[0:128, QVP_OFF + M_TILES : QVP_OFF + 2 * M_TILES] = vb
        mg[0:128, QVP_OFF + 2 * M_TILES : QVP_OFF + 2 * M_TILES + N_SL] = p[
            nsl
        ][None, :]
        mg[0:128, QVP_OFF + QVP_W - 1] = 1.0
        mg[0, TAIL_OFF : TAIL_OFF + 128] = qc[
            (M_TILES - 2) * 128 : (M_TILES - 1) * 128
        ]
        mg[0, TAIL_OFF + 128 : TAIL_OFF + 256] = qc[
            (M_TILES - 1) * 128 : M_TILES * 128
        ]
        mg[0, TAIL_OFF + 256 : TAIL_OFF + TAIL_W] = 1.0
        in_maps.append({"mg": mg, "mgd": DTm16})
    return in_maps


def gather_output(results, We_b):
    """results: list of per-core dicts with 'partial' [1, N_SL]."""
    out = np.concatenate(
        [np.asarray(results[c]["partial"])[0] for c in range(N_CORES)]
    ).astype(np.float64)[:, None]
    out += np.asarray(We_b, np.float64)[None, :]
    return out.astype(np.float32)


def kernel(
    X, D, R, Wx_w, Wx_b, Wd_w, Wd_b, We_w, We_b, encoding_ids, pos_encoding
):
    in_maps = make_in_maps(
        {
            "X": X,
            "D": D,
            "R": R,
            "Wx_w": Wx_w,
            "Wx_b": Wx_b,
            "Wd_w": Wd_w,
            "Wd_b": Wd_b,
            "We_w": We_w,
            "We_b": We_b,
            "encoding_ids": encoding_ids,
            "pos_encoding": pos_encoding,
        }
    )
    nc = _get_nc()
    res = run_bass_kernel_spmd(nc, in_maps, list(range(N_CORES)))
    return gather_output(res.results, We_b)
